# revision 17
# baseline (speedup 1.0000x reference)
"""Trainium2 Bass kernel for nn_MultiHeadAttention (x:[2,2048,512], 8 heads, d=64).

Sharding: 8 cores = 2 batches x 4 head-pairs. Each core computes the QKV
projection for its 2 heads, the attention, and a partial (row-split) O
projection. Host sums the 4 partials per batch and adds the output bias.

Per-core on-device layout (all contractions land on SBUF partitions):
  xT  [512, 2048]  = x[b].T                (host pre-transposed)
  QT  [128, 2048]  = wq.T @ xT             (head dims 2x64 on partitions)
  KT  [128, 2048]  = wk.T @ xT
  V   [2048, 128]  = xT.T @ wv             (natural; k-seq on partitions)
  PT  [2048, q]    = exp(scale * KT_h.T @ QT_h)   (scores^T, per head, bf16)
  avs [128, q]     = [V_h0|V_h1].T @ [PT_h0|PT_h1]  (one PSUM bank, both heads)
  sums[128, q]     = ones64.T @ PT_h  (denominators, PE-replicated per head half)
  Z   [128, 2048]  = avs * recip(sums)              (attn out, heads-transposed)
  out [2048, 512]  = Z.T @ wo                       (partial; host reduces)

PSUM accumulation-group trick: a bank supports one start/stop group, so each
shared bank is opened by an N=1 zero-weight "clear" matmul (start=True) and
closed by another (stop=True); all real matmuls accumulate with start=False
in any schedule order (per-element has_written handles first-write).
"""

import sys

import numpy as np

for _p in ("/opt/trn_rl_repo",):
    if _p not in sys.path:
        sys.path.insert(0, _p)

import concourse.bass as bass  # noqa: E402
import concourse.tile as tile  # noqa: E402
from concourse import bacc, mybir  # noqa: E402
from concourse.bass_utils import run_bass_kernel_spmd  # noqa: E402

EMBED = 512
NH = 8
HD = 64
S = 2048
B = 2
SCALE = HD ** -0.5
F32 = mybir.dt.float32

# float32r: single-pass matmul (tf32-like); cannot target dst partition != 0,
# so the AV/sums matmuls (which write to partition 64) use bf16 inputs.
MM_DT = mybir.dt.float32r
AV_DT = mybir.dt.bfloat16

N_KT = EMBED // 128   # 4 contraction k-tiles for the projections
N_QT = S // 512       # 4 q column tiles
N_ST = S // 128       # 16 seq tiles of 128


def build_nc():
    nc = bacc.Bacc("TRN2", target_bir_lowering=False, debug=False)

    xT_d = nc.dram_tensor("xT", [EMBED, S], MM_DT, kind="ExternalInput").ap()
    wq_d = nc.dram_tensor("wq", [EMBED, 128], MM_DT, kind="ExternalInput").ap()
    wk_d = nc.dram_tensor("wk", [EMBED, 128], MM_DT, kind="ExternalInput").ap()
    wv_d = nc.dram_tensor("wv", [EMBED, 128], MM_DT, kind="ExternalInput").ap()
    wo_d = nc.dram_tensor("wo", [128, EMBED], MM_DT, kind="ExternalInput").ap()
    out_d = nc.dram_tensor("out", [S, EMBED], F32, kind="ExternalOutput").ap()

    with tile.TileContext(nc) as tc:
        with (
            tc.tile_pool(name="persist", bufs=1) as persist,
            tc.tile_pool(name="pt_pool", bufs=4) as pt_pool,
            tc.tile_pool(name="norm", bufs=2) as norm_pool,
            tc.tile_pool(name="ostage", bufs=3) as ostage,
            tc.tile_pool(name="ps", bufs=2, space="PSUM") as ps_pool,
            tc.tile_pool(name="ps_avh0", bufs=2, space="PSUM") as ps_avh0_pool,
            tc.tile_pool(name="ps_avh1", bufs=2, space="PSUM") as ps_avh1_pool,
        ):
            # ---- load phase: small weights first so the first projection
            # matmuls only wait on xT tile arrivals; xT spread over queues ----
            wq_sb = persist.tile([128, N_KT, 128], MM_DT)
            wk_sb = persist.tile([128, N_KT, 128], MM_DT)
            wv_sb = persist.tile([128, N_KT, 128], MM_DT)
            for w_sb, w_d in ((wk_sb, wk_d), (wq_sb, wq_d), (wv_sb, wv_d)):
                for k in range(N_KT):
                    nc.sync.dma_start(out=w_sb[:, k, :], in_=w_d[k * 128:(k + 1) * 128, :])
            wo_sb = persist.tile([128, EMBED], MM_DT)
            nc.sync.dma_start(out=wo_sb, in_=wo_d)
            xT_sb = persist.tile([128, N_KT, S], MM_DT)  # [part, ktile, seq]
            dma_engines = (nc.sync, nc.scalar, nc.gpsimd, nc.sync)
            half = S // 2
            for k in range(N_KT):
                nc.sync.dma_start(
                    out=xT_sb[:, k, 0:half], in_=xT_d[k * 128:(k + 1) * 128, 0:half]
                )
                nc.scalar.dma_start(
                    out=xT_sb[:, k, half:S], in_=xT_d[k * 128:(k + 1) * 128, half:S]
                )


            # ---- qkv projections ----
            KT_sb = persist.tile([128, S], MM_DT)
            QT_sb = persist.tile([128, S], MM_DT)
            for w_sb, t_sb in ((wk_sb, KT_sb), (wq_sb, QT_sb)):
                for qt in range(N_QT):
                    qs = bass.ts(qt, 512)
                    ps = ps_pool.tile([128, 2, 512], F32, tag="ps")
                    for k in range(N_KT):
                        nc.tensor.matmul(
                            ps[:, 0, :],
                            w_sb[:, k, :],
                            xT_sb[:, k, qs],
                            start=(k == 0),
                            stop=(k == N_KT - 1),
                        )
                    nc.vector.tensor_copy(t_sb[:, qs], ps[:, 0, :])
            # V with a baked all-ones 65th column: the M=65 AV matmuls then
            # produce the softmax denominators in psum row 64 for free
            V_sb = persist.tile([128, N_ST, 2, HD + 1], AV_DT)
            nc.vector.memset(V_sb, 1.0)
            for st in range(N_ST):
                ps = ps_pool.tile([128, 2, 512], F32, tag="ps")
                for k in range(N_KT):
                    nc.tensor.matmul(
                        ps[:, 0, 0:128],
                        xT_sb[:, k, bass.ts(st, 128)],
                        wv_sb[:, k, :],
                        start=(k == 0),
                        stop=(k == N_KT - 1),
                    )
                nc.vector.tensor_copy(V_sb[:, st, 0, 0:HD], ps[:, 0, 0:64])
                nc.vector.tensor_copy(V_sb[:, st, 1, 0:HD], ps[:, 0, 64:128])

            # ---- attention (+ interleaved O-projection of the previous tile) ----
            Z_sb = persist.tile([128, S], MM_DT)  # normalized attn out^T, 2 heads

            def emit_oproj(src_qt):
                # O-projection matmuls for q range src_qt; emitted two chunks
                # into the next tile's k-loop so the PE never waits on the
                # normalize chain; psum rotates through the scores slots
                for mi in range(4):
                    m = 4 * src_qt + mi
                    po = ps_pool.tile([128, 512], F32, tag="ps")
                    nc.tensor.matmul(
                        po, Z_sb[:, bass.ts(m, 128)], wo_sb, start=True, stop=True,
                    )
                    ot = ostage.tile([128, 512], F32, tag="ot")
                    nc.vector.tensor_copy(ot, po)
                    nc.sync.dma_start(out=out_d[bass.ts(m, 128), :], in_=ot)

            for qt in range(N_QT):
                qs = bass.ts(qt, 512)
                av0 = ps_avh0_pool.tile([128, 512], F32, tag="avh0")
                av1 = ps_avh1_pool.tile([128, 512], F32, tag="avh1")
                for c in range(N_ST // 2):  # chunks of 2 k-subtiles
                    if c == 2 and qt > 0:
                        emit_oproj(qt - 1)
                    s0 = ps_pool.tile([128, 2, 512], F32, tag="ps")
                    s1 = ps_pool.tile([128, 2, 512], F32, tag="ps")
                    for j in range(2):
                        ks = 2 * c + j
                        kk = bass.ts(ks, 128)
                        # adjacent head-paired score matmuls: disjoint row
                        # groups (0:64 / 64:128) overlap on the PE array
                        nc.tensor.matmul(
                            s0[:, j, :], KT_sb[0:64, kk], QT_sb[0:64, qs],
                            start=True, stop=True,
                        )
                        nc.tensor.matmul(
                            s1[:, j, :], KT_sb[64:128, kk], QT_sb[64:128, qs],
                            start=True, stop=True,
                        )
                    pt0 = pt_pool.tile([128, 2, 512], AV_DT, tag="pt")
                    pt1 = pt_pool.tile([128, 2, 512], AV_DT, tag="pt")
                    nc.scalar.activation(
                        out=pt0, in_=s0, func=mybir.ActivationFunctionType.Exp,
                        scale=SCALE,
                    )
                    nc.scalar.activation(
                        out=pt1, in_=s1, func=mybir.ActivationFunctionType.Exp,
                        scale=SCALE,
                    )
                    for j in range(2):
                        ks = 2 * c + j
                        # M=65: rows 0:64 accumulate V^T @ PT, row 64 (ones
                        # column) accumulates the softmax denominators
                        nc.tensor.matmul(
                            av0[0:HD + 1, :], V_sb[:, ks, 0, :], pt0[:, j, :],
                            start=(ks == 0), stop=(ks == N_ST - 1),
                        )
                        nc.tensor.matmul(
                            av1[0:HD + 1, :], V_sb[:, ks, 1, :], pt1[:, j, :],
                            start=(ks == 0), stop=(ks == N_ST - 1),
                        )
                # normalize: shift-copy the denominator rows (psum row 64) to
                # partition 0, reciprocal, gpsimd-broadcast to 64 partitions,
                # then one multiply per head (h1's output write is
                # partition-shifted to rows 64:128 by the DVE)
                s_row = norm_pool.tile([1, 2, 512], F32, tag="s_row")
                nc.vector.tensor_copy(s_row[0:1, 0, :], av0[64:65, :])
                nc.vector.tensor_copy(s_row[0:1, 1, :], av1[64:65, :])
                r0_sb = norm_pool.tile([1, 2, 512], F32, tag="r0")
                nc.vector.reciprocal_approx_fast(
                    out=r0_sb[0:1, 0, :], in_=s_row[0:1, 0, :]
                )
                nc.vector.reciprocal_approx_fast(
                    out=r0_sb[0:1, 1, :], in_=s_row[0:1, 1, :]
                )
                rb_sb = norm_pool.tile([64, 2, 512], F32, tag="rb")
                nc.gpsimd.partition_broadcast(
                    out_ap=rb_sb[0:64, 0, :], in_ap=r0_sb[0:1, 0, :]
                )
                nc.gpsimd.partition_broadcast(
                    out_ap=rb_sb[0:64, 1, :], in_ap=r0_sb[0:1, 1, :]
                )
                nc.vector.tensor_mul(Z_sb[0:64, qs], av0[0:64, :], rb_sb[0:64, 0, :])
                nc.vector.tensor_mul(Z_sb[64:128, qs], av1[0:64, :], rb_sb[0:64, 1, :])
            emit_oproj(N_QT - 1)

    nc.compile()
    return nc


_NC = None


def _get_nc():
    global _NC
    if _NC is None:
        _NC = build_nc()
    return _NC


def make_in_maps(x, w_qkv, w_o):
    x = np.ascontiguousarray(np.asarray(x, dtype=np.float32))
    w_qkv = np.asarray(w_qkv, dtype=np.float32)
    w_o = np.asarray(w_o, dtype=np.float32)
    in_maps = []
    xTs = [np.ascontiguousarray(x[b].T) for b in range(B)]
    for c in range(8):
        b, g = c // 4, c % 4
        cols = slice(2 * g * HD, (2 * g + 2) * HD)
        in_maps.append({
            "xT": xTs[b],
            "wq": np.ascontiguousarray(w_qkv[:, :EMBED][:, cols]),
            "wk": np.ascontiguousarray(w_qkv[:, EMBED:2 * EMBED][:, cols]),
            "wv": np.ascontiguousarray(w_qkv[:, 2 * EMBED:][:, cols]),
            "wo": np.ascontiguousarray(w_o[cols, :]),
        })
    return in_maps


def combine(results, b_o):
    partials = np.stack([r["out"] for r in results])  # [8, S, EMBED]
    out = partials.reshape(B, 4, S, EMBED).sum(axis=1)
    return (out + np.asarray(b_o, dtype=np.float32)).astype(np.float32)


def kernel(x, w_qkv, w_o, b_o):
    nc = _get_nc()
    res = run_bass_kernel_spmd(nc, make_in_maps(x, w_qkv, w_o), core_ids=list(range(8)))
    return combine(res.results, b_o)


# revision 18
# speedup vs baseline: 1.0394x; 1.0394x over previous
"""Trainium2 Bass kernel for nn_MultiHeadAttention (x:[2,2048,512], 8 heads, d=64).

Sharding: 8 cores = 2 batches x 4 head-pairs. Each core computes the QKV
projection for its 2 heads, the attention, and a partial (row-split) O
projection. Host sums the 4 partials per batch and adds the output bias.

Per-core on-device layout (all contractions land on SBUF partitions):
  xT  [512, 2048]  = x[b].T                (host pre-transposed)
  QT  [128, 2048]  = wq.T @ xT             (head dims 2x64 on partitions)
  KT  [128, 2048]  = wk.T @ xT
  V   [2048, 128]  = xT.T @ wv             (natural; k-seq on partitions)
  PT  [2048, q]    = exp(scale * KT_h.T @ QT_h)   (scores^T, per head, bf16)
  avs [128, q]     = [V_h0|V_h1].T @ [PT_h0|PT_h1]  (one PSUM bank, both heads)
  sums[128, q]     = ones64.T @ PT_h  (denominators, PE-replicated per head half)
  Z   [128, 2048]  = avs * recip(sums)              (attn out, heads-transposed)
  out [2048, 512]  = Z.T @ wo                       (partial; host reduces)

PSUM accumulation-group trick: a bank supports one start/stop group, so each
shared bank is opened by an N=1 zero-weight "clear" matmul (start=True) and
closed by another (stop=True); all real matmuls accumulate with start=False
in any schedule order (per-element has_written handles first-write).
"""

import sys

import numpy as np

for _p in ("/opt/trn_rl_repo",):
    if _p not in sys.path:
        sys.path.insert(0, _p)

import concourse.bass as bass  # noqa: E402
import concourse.tile as tile  # noqa: E402
from concourse import bacc, mybir  # noqa: E402
from concourse.bass_utils import run_bass_kernel_spmd  # noqa: E402

EMBED = 512
NH = 8
HD = 64
S = 2048
B = 2
SCALE = HD ** -0.5
F32 = mybir.dt.float32

# float32r: single-pass matmul (tf32-like); cannot target dst partition != 0,
# so the AV/sums matmuls (which write to partition 64) use bf16 inputs.
MM_DT = mybir.dt.float32r
AV_DT = mybir.dt.bfloat16

N_KT = EMBED // 128   # 4 contraction k-tiles for the projections
N_QT = S // 512       # 4 q column tiles
N_ST = S // 128       # 16 seq tiles of 128


def build_nc():
    nc = bacc.Bacc("TRN2", target_bir_lowering=False, debug=False)

    xT_d = nc.dram_tensor("xT", [EMBED, S], MM_DT, kind="ExternalInput").ap()
    wq_d = nc.dram_tensor("wq", [EMBED, 128], MM_DT, kind="ExternalInput").ap()
    wk_d = nc.dram_tensor("wk", [EMBED, 128], MM_DT, kind="ExternalInput").ap()
    wv_d = nc.dram_tensor("wv", [EMBED, 128], MM_DT, kind="ExternalInput").ap()
    wo_d = nc.dram_tensor("wo", [128, EMBED], MM_DT, kind="ExternalInput").ap()
    out_d = nc.dram_tensor("out", [S, EMBED], F32, kind="ExternalOutput").ap()

    with tile.TileContext(nc) as tc:
        with (
            tc.tile_pool(name="persist", bufs=1) as persist,
            tc.tile_pool(name="pt_pool", bufs=4) as pt_pool,
            tc.tile_pool(name="norm", bufs=2) as norm_pool,
            tc.tile_pool(name="ostage", bufs=3) as ostage,
            tc.tile_pool(name="ps", bufs=2, space="PSUM") as ps_pool,
            tc.tile_pool(name="ps_avh0", bufs=2, space="PSUM") as ps_avh0_pool,
            tc.tile_pool(name="ps_avh1", bufs=2, space="PSUM") as ps_avh1_pool,
        ):
            # ---- load phase: few, large DMA descriptors (issue cost is
            # ~0.6us each); weights first so the first projection matmuls
            # only wait on the first xT tiles ----
            wq_sb = persist.tile([128, N_KT, 128], MM_DT)
            wk_sb = persist.tile([128, N_KT, 128], MM_DT)
            wv_sb = persist.tile([128, N_KT, 128], MM_DT)
            for w_sb, w_d in ((wk_sb, wk_d), (wq_sb, wq_d), (wv_sb, wv_d)):
                # one 3D descriptor: [part, ktile, col] <- [512, 128] dram
                nc.sync.dma_start(
                    out=w_sb,
                    in_=w_d.rearrange("(t p) m -> p t m", p=128),
                )
            wo_sb = persist.tile([128, EMBED], MM_DT)
            nc.sync.dma_start(out=wo_sb, in_=wo_d)
            xT_sb = persist.tile([128, N_KT, S], MM_DT)  # [part, ktile, seq]
            xT_r = xT_d.rearrange("(t p) s -> p t s", p=128)
            for k in range(N_KT):
                eng = nc.sync if k % 2 == 0 else nc.scalar
                eng.dma_start(out=xT_sb[:, k, :], in_=xT_r[:, k, :])


            # ---- qkv projections ----
            KT_sb = persist.tile([128, S], MM_DT)
            QT_sb = persist.tile([128, S], MM_DT)
            for w_sb, t_sb in ((wk_sb, KT_sb), (wq_sb, QT_sb)):
                for qt in range(N_QT):
                    qs = bass.ts(qt, 512)
                    ps = ps_pool.tile([128, 2, 512], F32, tag="ps")
                    for k in range(N_KT):
                        nc.tensor.matmul(
                            ps[:, 0, :],
                            w_sb[:, k, :],
                            xT_sb[:, k, qs],
                            start=(k == 0),
                            stop=(k == N_KT - 1),
                        )
                    nc.vector.tensor_copy(t_sb[:, qs], ps[:, 0, :])
            # V with a baked all-ones 65th column: the M=65 AV matmuls then
            # produce the softmax denominators in psum row 64 for free
            V_sb = persist.tile([128, N_ST, 2, HD + 1], AV_DT)
            nc.vector.memset(V_sb, 1.0)
            for st in range(N_ST):
                ps = ps_pool.tile([128, 2, 512], F32, tag="ps")
                for k in range(N_KT):
                    nc.tensor.matmul(
                        ps[:, 0, 0:128],
                        xT_sb[:, k, bass.ts(st, 128)],
                        wv_sb[:, k, :],
                        start=(k == 0),
                        stop=(k == N_KT - 1),
                    )
                nc.vector.tensor_copy(V_sb[:, st, 0, 0:HD], ps[:, 0, 0:64])
                nc.vector.tensor_copy(V_sb[:, st, 1, 0:HD], ps[:, 0, 64:128])

            # ---- attention (+ interleaved O-projection of the previous tile) ----
            Z_sb = persist.tile([128, S], MM_DT)  # normalized attn out^T, 2 heads

            def emit_oproj(src_qt):
                # O-projection matmuls for q range src_qt; emitted two chunks
                # into the next tile's k-loop so the PE never waits on the
                # normalize chain. One psum tile (alternating avh0 slot) is
                # reused serially by the 4 matmuls.
                po = ps_avh0_pool.tile([128, 512], F32, tag="avh0")
                for mi in range(4):
                    m = 4 * src_qt + mi
                    nc.tensor.matmul(
                        po, Z_sb[:, bass.ts(m, 128)], wo_sb, start=True, stop=True,
                    )
                    ot = ostage.tile([128, 512], F32, tag="ot")
                    nc.vector.tensor_copy(ot, po)
                    nc.sync.dma_start(out=out_d[bass.ts(m, 128), :], in_=ot)

            for qt in range(N_QT):
                qs = bass.ts(qt, 512)
                av0 = ps_avh0_pool.tile([128, 512], F32, tag="avh0")
                av1 = ps_avh1_pool.tile([128, 512], F32, tag="avh1")
                for c in range(N_ST // 2):  # chunks of 2 k-subtiles
                    if c == 2 and qt > 0:
                        emit_oproj(qt - 1)
                    s0 = ps_pool.tile([128, 2, 512], F32, tag="ps")
                    s1 = ps_pool.tile([128, 2, 512], F32, tag="ps")
                    for j in range(2):
                        ks = 2 * c + j
                        kk = bass.ts(ks, 128)
                        # adjacent head-paired score matmuls: disjoint row
                        # groups (0:64 / 64:128) overlap on the PE array
                        nc.tensor.matmul(
                            s0[:, j, :], KT_sb[0:64, kk], QT_sb[0:64, qs],
                            start=True, stop=True,
                        )
                        nc.tensor.matmul(
                            s1[:, j, :], KT_sb[64:128, kk], QT_sb[64:128, qs],
                            start=True, stop=True,
                        )
                    pt0 = pt_pool.tile([128, 2, 512], AV_DT, tag="pt")
                    pt1 = pt_pool.tile([128, 2, 512], AV_DT, tag="pt")
                    nc.scalar.activation(
                        out=pt0, in_=s0, func=mybir.ActivationFunctionType.Exp,
                        scale=SCALE,
                    )
                    nc.scalar.activation(
                        out=pt1, in_=s1, func=mybir.ActivationFunctionType.Exp,
                        scale=SCALE,
                    )
                    for j in range(2):
                        ks = 2 * c + j
                        # M=65: rows 0:64 accumulate V^T @ PT, row 64 (ones
                        # column) accumulates the softmax denominators
                        nc.tensor.matmul(
                            av0[0:HD + 1, :], V_sb[:, ks, 0, :], pt0[:, j, :],
                            start=(ks == 0), stop=(ks == N_ST - 1),
                        )
                        nc.tensor.matmul(
                            av1[0:HD + 1, :], V_sb[:, ks, 1, :], pt1[:, j, :],
                            start=(ks == 0), stop=(ks == N_ST - 1),
                        )
                # normalize: shift-copy the denominator rows (psum row 64) to
                # partition 0, reciprocal, gpsimd-broadcast to 64 partitions,
                # then one multiply per head (h1's output write is
                # partition-shifted to rows 64:128 by the DVE)
                s_row = norm_pool.tile([1, 2, 512], F32, tag="s_row")
                nc.vector.tensor_copy(s_row[0:1, 0, :], av0[64:65, :])
                nc.vector.tensor_copy(s_row[0:1, 1, :], av1[64:65, :])
                r0_sb = norm_pool.tile([1, 2, 512], F32, tag="r0")
                nc.vector.reciprocal_approx_fast(
                    out=r0_sb[0:1, 0, :], in_=s_row[0:1, 0, :]
                )
                nc.vector.reciprocal_approx_fast(
                    out=r0_sb[0:1, 1, :], in_=s_row[0:1, 1, :]
                )
                rb_sb = norm_pool.tile([64, 2, 512], F32, tag="rb")
                nc.gpsimd.partition_broadcast(
                    out_ap=rb_sb[0:64, 0, :], in_ap=r0_sb[0:1, 0, :]
                )
                nc.gpsimd.partition_broadcast(
                    out_ap=rb_sb[0:64, 1, :], in_ap=r0_sb[0:1, 1, :]
                )
                nc.vector.tensor_mul(Z_sb[0:64, qs], av0[0:64, :], rb_sb[0:64, 0, :])
                nc.vector.tensor_mul(Z_sb[64:128, qs], av1[0:64, :], rb_sb[0:64, 1, :])
            emit_oproj(N_QT - 1)

    nc.compile()
    return nc


_NC = None


def _get_nc():
    global _NC
    if _NC is None:
        _NC = build_nc()
    return _NC


def make_in_maps(x, w_qkv, w_o):
    x = np.ascontiguousarray(np.asarray(x, dtype=np.float32))
    w_qkv = np.asarray(w_qkv, dtype=np.float32)
    w_o = np.asarray(w_o, dtype=np.float32)
    in_maps = []
    xTs = [np.ascontiguousarray(x[b].T) for b in range(B)]
    for c in range(8):
        b, g = c // 4, c % 4
        cols = slice(2 * g * HD, (2 * g + 2) * HD)
        in_maps.append({
            "xT": xTs[b],
            "wq": np.ascontiguousarray(w_qkv[:, :EMBED][:, cols]),
            "wk": np.ascontiguousarray(w_qkv[:, EMBED:2 * EMBED][:, cols]),
            "wv": np.ascontiguousarray(w_qkv[:, 2 * EMBED:][:, cols]),
            "wo": np.ascontiguousarray(w_o[cols, :]),
        })
    return in_maps


def combine(results, b_o):
    partials = np.stack([r["out"] for r in results])  # [8, S, EMBED]
    out = partials.reshape(B, 4, S, EMBED).sum(axis=1)
    return (out + np.asarray(b_o, dtype=np.float32)).astype(np.float32)


def kernel(x, w_qkv, w_o, b_o):
    nc = _get_nc()
    res = run_bass_kernel_spmd(nc, make_in_maps(x, w_qkv, w_o), core_ids=list(range(8)))
    return combine(res.results, b_o)
